# revision 16
# baseline (speedup 1.0000x reference)
"""Trainium2 Bass kernel for GCNN message passing.

out[b] = relu((A @ x[b]) @ W + bias),  A sparse [N, N] from 800k edges.

Sharding (8 NeuronCores): core h owns dest rows [h*6272, (h+1)*6272) for all
4 batches. Host interleaves x into xcat[n] = x[:, n, :] (bf16, [N, 4*128])
so one gather index fetches a neighbor's features for all 4 batches.

The gather (gpsimd dma_gather) costs ~3us/call + ~7ns/index of Q7 time and
is the bottleneck resource, so the design minimizes gather calls and index
count:
  - dest rows are processed in 25 blocks of 256 rows; gathers span 2 blocks
    (13 spans x 2 calls: one per int16 index half, col < / >= 32768).
  - within each (block, half), edges are deduplicated per 128-row sub-block
    (same col -> one gathered slot; its S column carries both rows' vals).
  - slot layout per (block, half): [0, PA) sub-A slots, [PA, PA+nb) sub-B
    slots, 0-padded to T*128. PA and T are max'd over the 8 cores so the
    SPMD instruction schedule is identical on every core.

Device per block (segment-sum via matmul, 128-row sub-blocks):
  - scatter: agg_ps[128 rows, 4*128 (b,c)] += S_tile[128 slot, 128 row].T
    @ msgs[128 slot, 512], accumulated over the block's tiles. S tiles are
    host-built bf16 and streamed (matmul order: sub A tiles then sub B).
  - drain per sub-block: DVE copy agg->SBUF bf16, 4x PE transpose ->
    aggT[128 c, 4*128 (b,r)], DVE copy, W GEMM (lhsT=W[c,o]), ACT
    relu(.+bias[o]) -> ostage bf16, batched DMA to outT [128 o, 49, 512].
Host transposes/concatenates per-core outputs and casts to f32.
"""
import sys

import numpy as np

try:  # concourse (Bass) lives in the trn repo
    import concourse  # noqa: F401
except ImportError:  # pragma: no cover
    sys.path.insert(0, "/opt/trn_rl_repo")

import ml_dtypes

B, N, E, C = 4, 50000, 800000, 128
LAST_RESULTS = None  # BassKernelResults of the most recent kernel() call
P = 128
BR = 256            # rows per block
SUB = 128           # rows per scatter sub-block
NBLK = 25           # row-blocks per core (covers 6400 >= 6272 rows)
RH = 6272           # row stride between cores (8 * 6272 = 50176 >= N)
NSUB = RH // SUB    # 49 valid sub-blocks per core
NCORES = 8
SPLIT = 32768       # low/high column split for int16 gather indices
SPAN = 2            # row-blocks per gather call pair (head spans)
# span partition: big spans early (fewer calls), single-block spans at the
# end so the post-last-gather PE tail is one block, not two
SPANS = [list(range(j, j + 2)) for j in range(0, 20, 2)] + \
    [[b] for b in range(20, 25)]
OUT_DMA_SUBS = 2    # sub-blocks per output DMA
BC = B * C          # 512 feature cols in xcat


def _pack_idx(vals, n_slots):
    """dma_gather int16 index layout: index k at [k % 16, k // 16],
    replicated to 128 partitions; 0-padded. -> [128, n_slots // 16]"""
    buf = np.zeros(n_slots, np.int16)
    buf[:len(vals)] = vals
    tile16 = buf.reshape(n_slots // 16, 16).T
    return np.tile(tile16, (8, 1))


HUB_LO = 3200       # hub slots for cols < SPLIT (gathered once per core)
HUB_HI = 1408       # hub slots for cols >= SPLIT
HUB = HUB_LO + HUB_HI
HUB_TILES = HUB // P


def _preprocess(edge_row, edge_col, edge_vals):
    """Host: per-core gather index tables, S matrices, static schedule.

    Returns (sched, idx16 [8][128, TOTS//16], hubidx [8][128, HUB//16],
    smat [8][128, NMM*128] bf16). sched holds the static per-block
    structure shared by all cores.

    High-degree columns (per core) are "hubs": their features are gathered
    once into a persistent SBUF table; their edges contribute via HUB_TILES
    extra matmuls per sub-block instead of per-block gather slots.
    """
    # --- bucket edges by (core, block, half, sub); collect unique cols ---
    # edges[(h, blk, half)] = (cols, rows_local256, vals)
    buckets = {}
    hubidx = []
    hubmaps = []    # per core: dict col -> hub slot
    for h in range(NCORES):
        lo, hi = h * RH, min((h + 1) * RH, N)
        m = (edge_row >= lo) & (edge_row < hi)
        r, c, v = edge_row[m] - lo, edge_col[m], edge_vals[m]
        # per-core hub selection: top cols by degree, split by int16 half
        cols, cnt = np.unique(c, return_counts=True)
        is_hi = cols >= SPLIT
        lo_cols = cols[~is_hi][np.argsort(-cnt[~is_hi], kind="stable")]
        hi_cols = cols[is_hi][np.argsort(-cnt[is_hi], kind="stable")]
        hubs = np.zeros(HUB, np.int64)
        nlo = min(HUB_LO, len(lo_cols))
        nhi = min(HUB_HI, len(hi_cols))
        hubs[:nlo] = lo_cols[:nlo]
        hubs[HUB_LO:HUB_LO + nhi] = hi_cols[:nhi]
        hubmap = {int(cc): k for k, cc in enumerate(hubs[:nlo])}
        hubmap.update({int(cc): HUB_LO + k
                       for k, cc in enumerate(hubs[HUB_LO:HUB_LO + nhi])})
        hubmaps.append(hubmap)
        hv = hubs.copy()
        hv[HUB_LO:] -= SPLIT
        hubidx.append(np.concatenate(
            [_pack_idx(hv[:HUB_LO].astype(np.int16), HUB_LO),
             _pack_idx(hv[HUB_LO:].astype(np.int16), HUB_HI)], axis=1))

        ishub = np.isin(c, hubs[np.concatenate(
            [np.arange(nlo), HUB_LO + np.arange(nhi)])])
        # hub edges: (hub slot, global row) per core
        hub_slot = np.array([hubmap[int(cc)] for cc in c[ishub]], np.int64)
        buckets[(h, "hub")] = (hub_slot, r[ishub], v[ishub])
        r, c, v = r[~ishub], c[~ishub], v[~ishub]
        blk = r // BR
        half = (c >= SPLIT).astype(np.int8)
        for b in range(NBLK):
            mb = blk == b
            for hf in range(2):
                mm = mb & (half == hf)
                buckets[(h, b, hf)] = (c[mm], r[mm] - b * BR, v[mm])

    # pass 1: per (blk, half) unique-col counts per sub -> static PA, T
    # uniq[(h, blk, half, sub)] = (unique_cols, edge_slot_pos, rows, vals)
    uniq = {}
    PA = np.zeros((NBLK, 2), np.int64)
    NBmax = np.zeros((NBLK, 2), np.int64)
    for key, (c, r, v) in buckets.items():
        if len(key) != 3:
            continue
        h, b, hf = key
        sub = (r >= SUB).astype(np.int8)
        for s in range(2):
            ms = sub == s
            uc, inv = np.unique(c[ms], return_inverse=True)
            uniq[(h, b, hf, s)] = (uc, inv, r[ms] - s * SUB, v[ms])
            if s == 0:
                PA[b, hf] = max(PA[b, hf], len(uc))
            else:
                NBmax[b, hf] = max(NBmax[b, hf], len(uc))
    T = -(-(PA + NBmax) // P)          # tiles per (blk, half)
    assert np.all(T[:, :].sum(axis=1) > 0)

    # static matmul schedule per blk: list of (half, tile, sub), ordered by
    # (sub, half, tile) so each sub's PSUM accumulation group is consecutive
    sched_mm = []                      # [blk] -> list of (half, tile, sub)
    # half: 0 = low gather, 1 = high gather, 2 = hub table
    for b in range(NBLK):
        mm = []
        for s in range(2):
            if b * 2 + s < NSUB:
                for t in range(HUB_TILES):
                    mm.append((2, t, s))
            for hf in range(2):
                if T[b, hf] == 0:
                    continue
                pa, t_all = int(PA[b, hf]), int(T[b, hf])
                tb, rem = divmod(pa, P)
                for t in range(t_all):
                    if s == 0:
                        if t < tb or (t == tb and rem > 0):
                            mm.append((hf, t, 0))
                    else:
                        if NBmax[b, hf] == 0:
                            continue
                        if t > tb or (t == tb and rem > 0) or \
                           (t == tb and rem == 0):
                            # rem == 0: tile tb starts sub B exactly
                            if t * P < pa + NBmax[b, hf]:
                                mm.append((hf, t, 1))
        # drop sub-B matmuls for the invalid trailing sub (blk 24 sub 1)
        if b * 2 + 1 >= NSUB:
            mm = [x for x in mm if x[2] == 0]
        sched_mm.append(mm)

    # every valid (blk, sub) needs >= 1 matmul so its PSUM region is
    # initialized (start=True) before the drain reads it
    for b in range(NBLK):
        for s in range(2):
            if b * 2 + s >= NSUB:
                continue
            if not any(x[2] == s for x in sched_mm[b]):
                hf = 0 if T[b, 0] > 0 else 1
                if T[b, hf] == 0:
                    T[b, hf] = 1      # pad tile: zero idx, zero S
                sched_mm[b].append((hf, 0, s))

    # tile offsets within a span's msgs buffer: [blk0.lo, blk1.lo,
    # blk0.hi, blk1.hi]
    moff = np.zeros((NBLK, 2), np.int64)
    span_tiles = []
    for blks in SPANS:
        off = 0
        for hf in range(2):
            for b in blks:
                moff[b, hf] = off
                off += T[b, hf]
        span_tiles.append(off)

    # pass 2: per-core idx tables and S matrices in static order
    nmm = [len(m) for m in sched_mm]
    idx16 = []
    smat = []
    for h in range(NCORES):
        idx_parts = []
        for blks in SPANS:
            for hf in range(2):
                vals = []
                for b in blks:
                    seg = np.zeros(int(T[b, hf]) * P, np.int64)
                    ua, _, _, _ = uniq[(h, b, hf, 0)]
                    ub, _, _, _ = uniq[(h, b, hf, 1)]
                    seg[:len(ua)] = ua
                    seg[PA[b, hf]:PA[b, hf] + len(ub)] = ub
                    if hf:
                        seg[:len(ua)] -= SPLIT
                        seg[PA[b, hf]:PA[b, hf] + len(ub)] -= SPLIT
                    vals.append(seg)
                v = np.concatenate(vals) if vals else np.zeros(0, np.int64)
                if len(v):
                    idx_parts.append(_pack_idx(v.astype(np.int16), len(v)))
        idx16.append(np.concatenate(idx_parts, axis=1))

        stiles = []
        hub_slot, hub_r, hub_v = buckets[(h, "hub")]
        for b in range(NBLK):
            # dense S per (half): [T*P, 256] then slice per matmul
            sfull = {}
            for hf in range(2):
                sf = np.zeros((int(T[b, hf]) * P, BR), np.float32)
                for s in range(2):
                    uc, inv, rr, vv = uniq[(h, b, hf, s)]
                    base = 0 if s == 0 else int(PA[b, hf])
                    np.add.at(sf, (base + inv, s * SUB + rr), vv)
                sfull[hf] = sf
            mh = (hub_r >= b * BR) & (hub_r < (b + 1) * BR)
            sfh = np.zeros((HUB, BR), np.float32)
            np.add.at(sfh, (hub_slot[mh], hub_r[mh] - b * BR), hub_v[mh])
            sfull[2] = sfh
            for hf, t, s in sched_mm[b]:
                stiles.append(
                    sfull[hf][t * P:(t + 1) * P, s * SUB:(s + 1) * SUB])
        sm = np.concatenate(stiles, axis=1) if stiles else \
            np.zeros((P, 0), np.float32)
        # stiles entries are [P, SUB]; concat along cols -> [P, nmm*SUB]
        smat.append(sm.astype(ml_dtypes.bfloat16))

    sched = dict(PA=PA, T=T, NBmax=NBmax, sched_mm=sched_mm, moff=moff,
                 span_tiles=span_tiles, nmm=nmm)
    return sched, idx16, hubidx, smat


def _build_program(sched):
    import concourse.bacc as bacc
    import concourse.tile as tile
    from concourse import mybir
    from concourse._compat import get_trn_type

    T, PA = sched["T"], sched["PA"]
    NBmax = sched["NBmax"]
    sched_mm, moff = sched["sched_mm"], sched["moff"]
    span_tiles, nmm = sched["span_tiles"], sched["nmm"]
    tot_mm = int(np.sum(nmm))
    tot_idx16 = int(np.sum(T)) * P // 16

    f32 = mybir.dt.float32
    bf16 = mybir.dt.bfloat16
    i16 = mybir.dt.int16
    nc = bacc.Bacc(get_trn_type() or "TRN2", target_bir_lowering=False)

    x_d = nc.dram_tensor("xcat", [N, BC], bf16, kind="ExternalInput")
    idx_d = nc.dram_tensor("idx16", [P, tot_idx16], i16,
                           kind="ExternalInput")
    hubidx_d = nc.dram_tensor("hubidx", [P, HUB // 16], i16,
                              kind="ExternalInput")
    smat_d = nc.dram_tensor("smat", [P, tot_mm * SUB], bf16,
                            kind="ExternalInput")
    wt_d = nc.dram_tensor("wt", [C, C], bf16, kind="ExternalInput")
    bias_d = nc.dram_tensor("bias", [C, 1], f32, kind="ExternalInput")
    ident_d = nc.dram_tensor("ident", [P, P], bf16, kind="ExternalInput")
    out_d = nc.dram_tensor("outT", [C, NSUB, BC], bf16,
                           kind="ExternalOutput")

    with tile.TileContext(nc) as tc:
        with (
            tc.tile_pool(name="const", bufs=1) as const_pool,
            tc.tile_pool(name="meta", bufs=1) as meta_pool,
            tc.tile_pool(name="idxs", bufs=2) as idxs_pool,
            tc.tile_pool(name="msgs", bufs=2) as msgs_pool,
            tc.tile_pool(name="smat", bufs=2) as s_pool,
            tc.tile_pool(name="aggsb", bufs=1) as agg_pool,
            tc.tile_pool(name="aggTsb", bufs=1) as aggT_pool,
            tc.tile_pool(name="ostage", bufs=2) as ostage_pool,
            tc.tile_pool(name="psum_agg", bufs=2, space="PSUM") as psA,
            tc.tile_pool(name="psum_tr", bufs=2, space="PSUM") as psT,
            tc.tile_pool(name="psum_out", bufs=2, space="PSUM") as psO,
        ):
            wt_sb = const_pool.tile([C, C], bf16)
            bias_sb = const_pool.tile([C, 1], f32)
            ident_sb = const_pool.tile([P, P], bf16)

            max_span_n16 = max(span_tiles) * P // 16
            hubidx_sb = meta_pool.tile([P, HUB // 16], i16)
            # hubidx first: the hub gather is the head of the Q7 stream
            nc.sync.dma_start(out=hubidx_sb[:], in_=hubidx_d[:])
            nc.sync.dma_start(out=wt_sb[:], in_=wt_d[:])
            nc.sync.dma_start(out=bias_sb[:], in_=bias_d[:])
            nc.sync.dma_start(out=ident_sb[:], in_=ident_d[:])

            # hub feature table: gathered once, persists for the whole run
            hub_sb = const_pool.tile([P, HUB_TILES, BC], bf16)
            nc.gpsimd.dma_gather(
                out_ap=hub_sb[:, :HUB_LO // P, :],
                in_ap=x_d[:SPLIT, :],
                idxs_ap=hubidx_sb[:, :HUB_LO // 16],
                num_idxs=HUB_LO, num_idxs_reg=HUB_LO,
                elem_size=BC, single_packet=False,
            )
            nc.gpsimd.dma_gather(
                out_ap=hub_sb[:, HUB_LO // P:, :],
                in_ap=x_d[SPLIT:, :],
                idxs_ap=hubidx_sb[:, HUB_LO // 16:],
                num_idxs=HUB_HI, num_idxs_reg=HUB_HI,
                elem_size=BC, single_packet=False,
            )

            ostage = None
            mm_base = 0          # running matmul index into smat
            idx_off = 0          # running idx16 column offset
            for si, blks in enumerate(SPANS):
                ts = span_tiles[si]
                msgs = msgs_pool.tile([P, ts, BC], bf16)
                # this span's idx slice, double-buffered (Sync is idle)
                span_n16 = ts * P // 16
                idxspan = idxs_pool.tile([P, max_span_n16], i16)
                nc.sync.dma_start(
                    out=idxspan[:, :span_n16],
                    in_=idx_d[:, idx_off:idx_off + span_n16])
                idx_off += span_n16
                # two gather calls: low half then high half
                tile_cursor = 0
                for hf in range(2):
                    nt = int(sum(T[b, hf] for b in blks))
                    if nt == 0:
                        continue
                    nidx = nt * P
                    if si >= 2:
                        # exact-envelope trim: the last block's final tile is
                        # partially used; un-gathered slots keep stale (but
                        # finite) data from spans 0/1 and meet S = 0.
                        last = blks[-1]
                        used = (nt - int(T[last, hf])) * P +                             int(PA[last, hf] + NBmax[last, hf])
                        nidx = min(nidx, -(-used // 16) * 16)
                    nc.gpsimd.dma_gather(
                        out_ap=msgs[:, tile_cursor:tile_cursor + nt, :],
                        in_ap=x_d[:SPLIT, :] if hf == 0 else x_d[SPLIT:, :],
                        idxs_ap=idxspan[:, tile_cursor * 8:
                                        tile_cursor * 8 + nidx // 16],
                        num_idxs=nidx,
                        num_idxs_reg=nidx,
                        elem_size=BC,
                        single_packet=False,
                    )
                    tile_cursor += nt

                for b in blks:
                    s_sb = s_pool.tile([P, max(nmm[b], 1) * SUB], bf16)
                    nc.sync.dma_start(
                        out=s_sb[:, :nmm[b] * SUB],
                        in_=smat_d[:, mm_base * SUB:
                                   (mm_base + nmm[b]) * SUB])
                    agg_ps = psA.tile([P, 2 * BC], f32)
                    seen = {}
                    mmlist = sched_mm[b]
                    for k, (hf, t, s) in enumerate(mmlist):
                        first = s not in seen
                        seen[s] = True
                        last = all(x[2] != s for x in mmlist[k + 1:])
                        rhs = (hub_sb[:, t, :] if hf == 2 else
                               msgs[:, int(moff[b, hf]) + t, :])
                        nc.tensor.matmul(
                            out=agg_ps[:, s * BC:(s + 1) * BC],
                            lhsT=s_sb[:, k * SUB:(k + 1) * SUB],
                            rhs=rhs,
                            start=first, stop=last,
                        )
                    mm_base += nmm[b]

                    for s in range(2):
                        g = b * 2 + s
                        if g >= NSUB:
                            continue
                        aggsb = agg_pool.tile([P, BC], bf16)
                        nc.vector.tensor_copy(
                            out=aggsb[:], in_=agg_ps[:, s * BC:(s + 1) * BC])
                        aggT_ps = psT.tile([P, BC], bf16)
                        for bb in range(B):
                            nc.tensor.transpose(
                                out=aggT_ps[:, bb * C:(bb + 1) * C],
                                in_=aggsb[:, bb * C:(bb + 1) * C],
                                identity=ident_sb[:],
                            )
                        aggTsb = aggT_pool.tile([P, BC], bf16)
                        nc.vector.tensor_copy(out=aggTsb[:], in_=aggT_ps[:])
                        outT_ps = psO.tile([P, BC], f32)
                        nc.tensor.matmul(
                            out=outT_ps[:], lhsT=wt_sb[:], rhs=aggTsb[:],
                            start=True, stop=True)
                        if g % OUT_DMA_SUBS == 0:
                            ostage = ostage_pool.tile(
                                [P, OUT_DMA_SUBS, BC], bf16)
                        nc.scalar.activation(
                            out=ostage[:, g % OUT_DMA_SUBS, :],
                            in_=outT_ps[:],
                            func=mybir.ActivationFunctionType.Relu,
                            bias=bias_sb[:, :1], scale=1.0,
                        )
                        if g % OUT_DMA_SUBS == OUT_DMA_SUBS - 1 or \
                           g == NSUB - 1:
                            glo = (g // OUT_DMA_SUBS) * OUT_DMA_SUBS
                            nsub = g - glo + 1
                            nc.sync.dma_start(
                                out=out_d[:, glo:glo + nsub, :],
                                in_=ostage[:, :nsub, :],
                            )
    return nc


def _ensure_ntff_hook_importable():
    """bass_utils imports antenv.axon_hooks when BASS_TRACE is set; this
    image lacks that module. Provide a null hook so tracing degrades
    gracefully instead of crashing."""
    import types

    try:
        import antenv.axon_hooks  # noqa: F401
        return
    except ImportError:
        pass
    mod = types.ModuleType("antenv.axon_hooks")
    mod.get_axon_ntff_profile_hook = lambda: None
    mod.set_axon_ntff_profile_hook = lambda h: None
    sys.modules["antenv.axon_hooks"] = mod
    try:
        import antenv
        antenv.axon_hooks = mod
    except ImportError:
        pass


def kernel(x, edge_row, edge_col, edge_vals, W, b):
    _ensure_ntff_hook_importable()
    from concourse.bass_utils import run_bass_kernel_spmd

    x = np.asarray(x, np.float32)
    edge_row = np.asarray(edge_row, np.int32)
    edge_col = np.asarray(edge_col, np.int32)
    edge_vals = np.asarray(edge_vals, np.float32)
    W = np.asarray(W, np.float32)
    b = np.asarray(b, np.float32)

    sched, idx16, hubidx, smat = _preprocess(edge_row, edge_col, edge_vals)
    nc = _build_program(sched)
    nc.compile()

    # xcat[n] = x[:, n, :] flattened -> [N, 4*128] bf16
    xcat = np.ascontiguousarray(
        x.transpose(1, 0, 2).reshape(N, B * C)).astype(ml_dtypes.bfloat16)
    wt = W.astype(ml_dtypes.bfloat16)
    ident = np.eye(P, dtype=ml_dtypes.bfloat16)
    in_maps = []
    for h in range(NCORES):
        in_maps.append({
            "xcat": xcat,
            "idx16": idx16[h],
            "hubidx": hubidx[h],
            "smat": smat[h],
            "wt": wt,
            "bias": np.ascontiguousarray(b[:, None]),
            "ident": ident,
        })

    res = run_bass_kernel_spmd(nc, in_maps, list(range(NCORES)))
    global LAST_RESULTS
    LAST_RESULTS = res

    out = np.empty((B, N, C), np.float32)
    for h in range(NCORES):
        lo, hi = h * RH, min((h + 1) * RH, N)
        o = res.results[h]["outT"].astype(np.float32)   # [C, NSUB, 4*128]
        # o[c, g, bb*128 + r] = out[bb, lo + g*128 + r, c]
        o = o.reshape(C, NSUB, B, SUB).transpose(2, 1, 3, 0) \
             .reshape(B, NSUB * SUB, C)
        out[:, lo:hi] = o[:, :hi - lo]
    return out
